# revision 34
# baseline (speedup 1.0000x reference)
"""Single-head attention (B=4, S=4096, E=1024, D=64) on 8 TRN2 NeuronCores.

Sharding: data-parallel over (batch, query-half): core c handles batch
b = c // 2 and query rows [h*2048, (h+1)*2048) with h = c % 2. Each core
computes Q for its own 2048 rows and K/V for the full 4096 rows of its batch
(inputs are shipped host-pretransposed per half, so no duplicated DMA).

Per-core dataflow (TensorE matmuls in bf16 — fp32/fp32r matmuls run the PE
at half clock; fp32 accumulation in PSUM). Projections pack TWO weight
matrices into one 128-wide stationary operand:
  qk [128, 2048] = [K^T_own; Q^T_own]     (pass A, lhsT = [WkT | WqT])
  kv [128, 2048] = [K^T_oth; V^T_oth]     (pass B, lhsT = [WkT | WvT])
  vt [65, 2048]  = V^T_own + ones row      (pass C, lhsT = WvT)
Q^T / V^T_oth are then shifted to base partition 0 by SBUF-to-SBUF DMAs
(matmul operands must share a base partition).
  scores^T[k, q] = K^T.T @ Q^T -> exp on ScalarE (scale folded) -> P bf16
  attn^T[65, q] += V_aug.T @ P   (row 64 accumulates softmax denominators)
  output = attn^T with denominators; host transposes + normalizes.

The attention runs as TWO passes over q (1024 columns each): the attn
accumulator then fits 2 PSUM banks, freeing a third scores slot (PSUM slot
contention paced the single-pass version), and pass 0's output ships
mid-kernel. Warm-up and "heater" transposes keep the PE HAM clock at
2.4 GHz wherever real PE work alone would leave periodic idle gaps.
"""

import numpy as np

B, S, E, D = 4, 4096, 1024, 64
HALF = S // 2
N_CORES = 8
SCALE = 1.0 / np.sqrt(D)

NE = E // 128  # 8 e-tiles
NKT = S // 128  # 32 k-tiles
N_WARM = 60  # PE warm-up transposes covering the Tile preamble + DMA wait

_CACHE = {}


def _build():
    if "nc" in _CACHE:
        return _CACHE["nc"]

    from contextlib import ExitStack

    import concourse.bacc as bacc
    import concourse.tile as tile
    from concourse import mybir
    from concourse.masks import make_identity

    FP32 = mybir.dt.float32
    BF16 = mybir.dt.bfloat16
    Exp = mybir.ActivationFunctionType.Exp

    nc = bacc.Bacc(
        "TRN2", target_bir_lowering=False, debug=False, num_devices=N_CORES
    )

    xt_q_d = nc.dram_tensor("xt_q", [E, HALF], BF16, kind="ExternalInput").ap()
    xt_o_d = nc.dram_tensor("xt_o", [E, HALF], BF16, kind="ExternalInput").ap()
    wt_d = nc.dram_tensor("wt", [E, 320], BF16, kind="ExternalInput").ap()
    out_d = nc.dram_tensor("out", [D + 1, HALF], FP32, kind="ExternalOutput").ap()

    with tile.TileContext(nc) as tc, ExitStack() as ctx:
        const = ctx.enter_context(tc.tile_pool(name="const", bufs=1))
        big = ctx.enter_context(tc.tile_pool(name="big", bufs=1))
        pp = ctx.enter_context(tc.tile_pool(name="pp", bufs=6))
        psA = ctx.enter_context(tc.tile_pool(name="psA", bufs=3, space="PSUM"))
        psB = ctx.enter_context(tc.tile_pool(name="psB", bufs=1, space="PSUM"))

        identB = const.tile([128, 128], BF16)
        make_identity(nc, identB)

        xt = big.tile([128, NE, S], BF16)  # x^T; cols [0, HALF) = own q-rows
        wt = big.tile([128, NE, 320], BF16)  # [WkT|WqT | WkT|WvT | WvT]
        qk = big.tile([128, HALF], BF16)  # rows 0-63 K^T own, 64-127 Q^T own
        kv = big.tile([128, HALF], BF16)  # rows 0-63 K^T oth, 64-127 V^T oth
        qts = big.tile([64, HALF], BF16)  # Q^T shifted to base partition 0
        vto = big.tile([64, HALF], BF16)  # V^T other shifted to base part. 0
        vt = big.tile([65, HALF], BF16)  # V^T own; row 64 = ones
        vn = big.tile([128, NKT, D + 1], BF16)  # V natural + ones column
        att_sb = big.tile([65, HALF], FP32)  # attn^T + denominator row

        # --- PE warm-up: keep HAM at full clock through the preamble ---
        warm = psA.tile([128, 1024], BF16, tag="ps")
        for _ in range(N_WARM):
            nc.tensor.transpose(
                out=warm[0:128, 0:128], in_=identB[:, :], identity=identB[:, :]
            )

        # --- input DMAs. One dma_start ~= one DMA queue, so split per
        # e-tile. Priority: wt, own cols 0:1024 (gates pass A / first exp),
        # then other half (B lumps, ~iter 9+), then own cols 1024:2048
        # (A2/A3 lumps, ~iter 17+). sync stays free-ish for the shifts.
        nc.sync.dma_start(out=wt[:, :, :], in_=wt_d.rearrange("(t p) d -> p t d", p=128))
        for et in range(NE):
            eng = [nc.sync, nc.scalar][et % 2]
            eng.dma_start(
                out=xt[:, et, 0:1024],
                in_=xt_q_d[et * 128 : (et + 1) * 128, 0:1024],
            )
        for et in range(NE):
            nc.gpsimd.dma_start(
                out=xt[:, et, 1024:2048],
                in_=xt_q_d[et * 128 : (et + 1) * 128, 1024:2048],
            )
        for et in range(NE):
            nc.gpsimd.dma_start(
                out=xt[:, et, HALF:S],
                in_=xt_o_d[et * 128 : (et + 1) * 128, :],
            )

        nc.vector.memset(vt[64:65, :], 1.0)

        # one packed projection half-chunk of 512 cols
        def proj_half(w0, wm, dst, src_x0, d0):
            acc = psA.tile([128, 1024], FP32, tag="ps")
            for et in range(NE):
                nc.tensor.matmul(
                    out=acc[0:wm, 0:512],
                    lhsT=wt[:, et, w0 : w0 + wm],
                    rhs=xt[:, et, src_x0 : src_x0 + 512],
                    start=(et == 0),
                    stop=(et == NE - 1),
                )
            nc.vector.tensor_copy(out=dst[:, d0 : d0 + 512], in_=acc[0:wm, 0:512])

        def shift(dst, src, d0):
            nc.sync.dma_start(
                out=dst[:, d0 : d0 + 512], in_=src[64:128, d0 : d0 + 512]
            )

        def v_transpose(k):
            tp = psA.tile([128, 1024], BF16, tag="ps")
            if k < 16:  # own half: vt carries the ones row
                nc.tensor.transpose(
                    out=tp[0:128, 0:65],
                    in_=vt[:, k * 128 : (k + 1) * 128],
                    identity=identB[0:65, 0:65],
                )
                nc.vector.tensor_copy(out=vn[:, k, :], in_=tp[0:128, 0:65])
            else:  # other half: V^T shifted into vto (base partition 0)
                j = k - 16
                nc.tensor.transpose(
                    out=tp[0:128, 0:64],
                    in_=vto[:, j * 128 : (j + 1) * 128],
                    identity=identB[0:64, 0:64],
                )
                nc.vector.memset(vn[:, k, D : D + 1], 1.0)
                nc.vector.tensor_copy(out=vn[:, k, 0:D], in_=tp[0:128, 0:64])

        # --- prologue: pass A halves 0-1 (K^T + Q^T own, q-cols 0:1024) ---
        for hh in range(2):
            proj_half(0, 128, qk, hh * 512, hh * 512)
            shift(qts, qk, hh * 512)

        # side-slot schedule for pass 0: iter k -> (kind, half-index)
        SIDE = {
            1: ("C", 0), 3: ("C", 1), 5: ("A", 2), 7: ("C", 2),
            9: ("C", 3), 11: ("A", 3), 13: ("B", 0), 17: ("B", 1),
            21: ("B", 2), 25: ("B", 3),
        }

        def side_work(k):
            s = SIDE.get(k)
            if s is not None:
                kind, hh = s
                if kind == "A":
                    proj_half(0, 128, qk, hh * 512, hh * 512)
                    shift(qts, qk, hh * 512)
                elif kind == "C":
                    proj_half(256, 64, vt[0:64, :], hh * 512, hh * 512)
                else:
                    proj_half(128, 128, kv, HALF + hh * 512, hh * 512)
                    shift(vto, kv, hh * 512)
            if k == 2:
                for j in range(4):
                    v_transpose(j)
            elif k >= 3 and k + 1 < NKT:
                v_transpose(k + 1)

        out_engs = [nc.sync, nc.gpsimd]

        # --- two q-passes of 1024 columns each ---
        for ps in range(2):
            att_ps = psB.tile([128, 1024], FP32)
            p_tiles = {}

            for k in range(NKT):
                if ps == 0:
                    side_work(k)
                if k < 16:
                    klhs = qk[0:64, k * 128 : (k + 1) * 128]
                else:
                    klhs = kv[0:64, (k - 16) * 128 : (k - 15) * 128]

                sc = psA.tile([128, 1024], FP32, tag="ps")
                if ps == 1 and k >= 2:
                    # heater: keeps the PE HAM clock warm; result is
                    # overwritten by the start=True scores matmul below.
                    for _ in range(2):
                        nc.tensor.transpose(
                            out=sc[:, 0:64].bitcast(BF16),
                            in_=identB[:, :],
                            identity=identB[:, :],
                        )
                for c in range(2):
                    q0 = ps * 1024 + c * 512
                    nc.tensor.matmul(
                        out=sc[:, c * 512 : (c + 1) * 512],
                        lhsT=klhs,
                        rhs=qts[:, q0 : q0 + 512],
                        start=True,
                        stop=True,
                    )
                p = pp.tile([128, 1024], BF16)
                nc.scalar.activation(out=p[:, :], in_=sc[:, :], func=Exp, scale=SCALE)
                p_tiles[k] = p

                if k >= 2:
                    _attn(nc, att_ps, vn, p_tiles, k - 2)

            _attn(nc, att_ps, vn, p_tiles, NKT - 2)
            _attn(nc, att_ps, vn, p_tiles, NKT - 1)

            # ship this pass's attn^T + denominators (host normalizes)
            for c in range(2):
                cols = slice(ps * 1024 + c * 512, ps * 1024 + (c + 1) * 512)
                pcols = slice(c * 512, (c + 1) * 512)
                nc.vector.tensor_copy(out=att_sb[:, cols], in_=att_ps[0:65, pcols])
                out_engs[c].dma_start(out=out_d[:, cols], in_=att_sb[:, cols])

    nc.compile()
    _CACHE["nc"] = nc
    return nc


def _attn(nc, att_ps, vn, p_tiles, k):
    p = p_tiles.pop(k)
    for c in range(2):
        nc.tensor.matmul(
            out=att_ps[0:65, c * 512 : (c + 1) * 512],
            lhsT=vn[:, k, :],
            rhs=p[:, c * 512 : (c + 1) * 512],
            start=(k == 0),
            stop=(k == NKT - 1),
            skip_group_check=True,
        )


def _make_in_maps(x, Wq, Wk, Wv):
    import ml_dtypes

    bf16 = ml_dtypes.bfloat16
    xT = np.ascontiguousarray(x.transpose(0, 2, 1)).astype(bf16)  # [B, E, S]
    wt = np.concatenate(
        [Wk.T, Wq.T, Wk.T, Wv.T, Wv.T], axis=1
    ).astype(bf16)  # [E, 320]
    in_maps = []
    for c in range(N_CORES):
        b, h = divmod(c, 2)
        in_maps.append(
            {
                "xt_q": np.ascontiguousarray(xT[b, :, h * HALF : (h + 1) * HALF]),
                "xt_o": np.ascontiguousarray(
                    xT[b, :, (1 - h) * HALF : (2 - h) * HALF]
                ),
                "wt": wt,
            }
        )
    return in_maps


def _run(x, Wq, Wk, Wv, trace=False):
    from concourse.bass_utils import run_bass_kernel_spmd

    nc = _build()
    in_maps = _make_in_maps(x, Wq, Wk, Wv)
    res = run_bass_kernel_spmd(
        nc, in_maps, core_ids=list(range(N_CORES)), trace=trace
    )
    out = np.empty((B, S, D), dtype=np.float32)
    for c in range(N_CORES):
        b, h = divmod(c, 2)
        att = res.results[c]["out"]  # [65, HALF]: attn^T rows + denom row
        out[b, h * HALF : (h + 1) * HALF, :] = (att[0:D] / att[D : D + 1]).T
    return out, res


def kernel(x, Wq, Wk, Wv):
    out, _ = _run(
        np.asarray(x, dtype=np.float32),
        np.asarray(Wq, dtype=np.float32),
        np.asarray(Wk, dtype=np.float32),
        np.asarray(Wv, dtype=np.float32),
    )
    return out


# revision 36
# speedup vs baseline: 1.0261x; 1.0261x over previous
"""Single-head attention (B=4, S=4096, E=1024, D=64) on 8 TRN2 NeuronCores.

Sharding: data-parallel over (batch, query-half): core c handles batch
b = c // 2 and query rows [h*2048, (h+1)*2048) with h = c % 2. Each core
computes Q for its own 2048 rows and K/V for the full 4096 rows of its batch
(inputs are shipped host-pretransposed per half, so no duplicated DMA).

Per-core dataflow (TensorE matmuls in bf16 — fp32/fp32r matmuls run the PE
at half clock; fp32 accumulation in PSUM). Projections pack TWO weight
matrices into one 128-wide stationary operand:
  qk [128, 2048] = [K^T_own; Q^T_own]     (pass A, lhsT = [WkT | WqT])
  kv [128, 2048] = [K^T_oth; V^T_oth]     (pass B, lhsT = [WkT | WvT])
  vt [65, 2048]  = V^T_own + ones row      (pass C, lhsT = WvT)
Q^T / V^T_oth are then shifted to base partition 0 by SBUF-to-SBUF DMAs
(matmul operands must share a base partition).
  scores^T[k, q] = K^T.T @ Q^T -> exp on ScalarE (scale folded) -> P bf16
  attn^T[65, q] += V_aug.T @ P   (row 64 accumulates softmax denominators)
  output = attn^T with denominators; host transposes + normalizes.

The attention runs as TWO passes over q (1024 columns each): the attn
accumulator then fits 2 PSUM banks, freeing a third scores slot (PSUM slot
contention paced the single-pass version), and pass 0's output ships
mid-kernel. Warm-up and "heater" transposes keep the PE HAM clock at
2.4 GHz wherever real PE work alone would leave periodic idle gaps.
"""

import numpy as np

B, S, E, D = 4, 4096, 1024, 64
HALF = S // 2
N_CORES = 8
SCALE = 1.0 / np.sqrt(D)

NE = E // 128  # 8 e-tiles
NKT = S // 128  # 32 k-tiles
N_WARM = 60  # PE warm-up transposes covering the Tile preamble + DMA wait

_CACHE = {}


def _build():
    if "nc" in _CACHE:
        return _CACHE["nc"]

    from contextlib import ExitStack

    import concourse.bacc as bacc
    import concourse.tile as tile
    from concourse import mybir
    from concourse.masks import make_identity

    FP32 = mybir.dt.float32
    BF16 = mybir.dt.bfloat16
    Exp = mybir.ActivationFunctionType.Exp

    nc = bacc.Bacc(
        "TRN2", target_bir_lowering=False, debug=False, num_devices=N_CORES
    )

    xt_q_d = nc.dram_tensor("xt_q", [E, HALF], BF16, kind="ExternalInput").ap()
    xt_o_d = nc.dram_tensor("xt_o", [E, HALF], BF16, kind="ExternalInput").ap()
    wt_d = nc.dram_tensor("wt", [E, 320], BF16, kind="ExternalInput").ap()
    out_d = nc.dram_tensor("out", [D + 1, HALF], FP32, kind="ExternalOutput").ap()

    with tile.TileContext(nc) as tc, ExitStack() as ctx:
        const = ctx.enter_context(tc.tile_pool(name="const", bufs=1))
        big = ctx.enter_context(tc.tile_pool(name="big", bufs=1))
        pp = ctx.enter_context(tc.tile_pool(name="pp", bufs=6))
        psA = ctx.enter_context(tc.tile_pool(name="psA", bufs=3, space="PSUM"))
        psB = ctx.enter_context(tc.tile_pool(name="psB", bufs=1, space="PSUM"))

        identB = const.tile([128, 128], BF16)
        make_identity(nc, identB)

        xt = big.tile([128, NE, S], BF16)  # x^T; cols [0, HALF) = own q-rows
        wt = big.tile([128, NE, 320], BF16)  # [WkT|WqT | WkT|WvT | WvT]
        qk = big.tile([128, HALF], BF16)  # rows 0-63 K^T own, 64-127 Q^T own
        kv = big.tile([128, HALF], BF16)  # rows 0-63 K^T oth, 64-127 V^T oth
        qts = big.tile([64, HALF], BF16)  # Q^T shifted to base partition 0
        vto = big.tile([64, HALF], BF16)  # V^T other shifted to base part. 0
        vt = big.tile([65, HALF], BF16)  # V^T own; row 64 = ones
        vn = big.tile([128, NKT, D + 1], BF16)  # V natural + ones column
        att_sb = big.tile([65, HALF], FP32)  # attn^T + denominator row

        # --- PE warm-up: keep HAM at full clock through the preamble ---
        warm = psA.tile([128, 1024], BF16, tag="ps")
        for _ in range(N_WARM):
            nc.tensor.transpose(
                out=warm[0:128, 0:128], in_=identB[:, :], identity=identB[:, :]
            )

        # --- input DMAs. One dma_start ~= one DMA queue, so split per
        # e-tile. Priority: wt, own cols 0:1024 (gates pass A / first exp),
        # then other half (B lumps, ~iter 9+), then own cols 1024:2048
        # (A2/A3 lumps, ~iter 17+). sync stays free-ish for the shifts.
        nc.sync.dma_start(out=wt[:, :, :], in_=wt_d.rearrange("(t p) d -> p t d", p=128))
        # cols 0:1024 (gates the first exp): spread across all three
        # DMA-capable engines; gpsimd's SWDGE has multiple queues.
        first_engs = [nc.gpsimd, nc.sync, nc.gpsimd, nc.scalar,
                      nc.gpsimd, nc.sync, nc.gpsimd, nc.scalar]
        for et in range(NE):
            first_engs[et].dma_start(
                out=xt[:, et, 0:1024],
                in_=xt_q_d[et * 128 : (et + 1) * 128, 0:1024],
            )
        for et in range(NE):
            nc.gpsimd.dma_start(
                out=xt[:, et, 1024:2048],
                in_=xt_q_d[et * 128 : (et + 1) * 128, 1024:2048],
            )
        for et in range(NE):
            eng = [nc.gpsimd, nc.sync][et % 2]
            eng.dma_start(
                out=xt[:, et, HALF:S],
                in_=xt_o_d[et * 128 : (et + 1) * 128, :],
            )

        nc.vector.memset(vt[64:65, :], 1.0)

        # one packed projection half-chunk of 512 cols
        def proj_half(w0, wm, dst, src_x0, d0):
            acc = psA.tile([128, 1024], FP32, tag="ps")
            for et in range(NE):
                nc.tensor.matmul(
                    out=acc[0:wm, 0:512],
                    lhsT=wt[:, et, w0 : w0 + wm],
                    rhs=xt[:, et, src_x0 : src_x0 + 512],
                    start=(et == 0),
                    stop=(et == NE - 1),
                )
            nc.vector.tensor_copy(out=dst[:, d0 : d0 + 512], in_=acc[0:wm, 0:512])

        def shift(dst, src, d0):
            # scalar's DMA queue is otherwise idle until the exps begin,
            # so the shifts never wait behind bulk input pieces.
            nc.scalar.dma_start(
                out=dst[:, d0 : d0 + 512], in_=src[64:128, d0 : d0 + 512]
            )

        def v_transpose(k):
            tp = psA.tile([128, 1024], BF16, tag="ps")
            if k < 16:  # own half: vt carries the ones row
                nc.tensor.transpose(
                    out=tp[0:128, 0:65],
                    in_=vt[:, k * 128 : (k + 1) * 128],
                    identity=identB[0:65, 0:65],
                )
                nc.vector.tensor_copy(out=vn[:, k, :], in_=tp[0:128, 0:65])
            else:  # other half: V^T shifted into vto (base partition 0)
                j = k - 16
                nc.tensor.transpose(
                    out=tp[0:128, 0:64],
                    in_=vto[:, j * 128 : (j + 1) * 128],
                    identity=identB[0:64, 0:64],
                )
                nc.vector.memset(vn[:, k, D : D + 1], 1.0)
                nc.vector.tensor_copy(out=vn[:, k, 0:D], in_=tp[0:128, 0:64])

        # --- prologue: pass A halves 0-1 (K^T + Q^T own, q-cols 0:1024) ---
        for hh in range(2):
            proj_half(0, 128, qk, hh * 512, hh * 512)
            shift(qts, qk, hh * 512)

        # side-slot schedule for pass 0: iter k -> (kind, half-index)
        SIDE = {
            1: ("C", 0), 3: ("C", 1), 5: ("A", 2), 7: ("C", 2),
            9: ("C", 3), 11: ("A", 3), 13: ("B", 0), 17: ("B", 1),
            21: ("B", 2), 25: ("B", 3),
        }

        def side_work(k):
            s = SIDE.get(k)
            if s is not None:
                kind, hh = s
                if kind == "A":
                    proj_half(0, 128, qk, hh * 512, hh * 512)
                    shift(qts, qk, hh * 512)
                elif kind == "C":
                    proj_half(256, 64, vt[0:64, :], hh * 512, hh * 512)
                else:
                    proj_half(128, 128, kv, HALF + hh * 512, hh * 512)
                    shift(vto, kv, hh * 512)
            if k == 2:
                for j in range(4):
                    v_transpose(j)
            elif k >= 3 and k + 1 < NKT:
                v_transpose(k + 1)

        out_engs = [nc.sync, nc.gpsimd]

        # --- two q-passes of 1024 columns each ---
        for ps in range(2):
            att_ps = psB.tile([128, 1024], FP32)
            p_tiles = {}

            for k in range(NKT):
                if ps == 0:
                    side_work(k)
                if k < 16:
                    klhs = qk[0:64, k * 128 : (k + 1) * 128]
                else:
                    klhs = kv[0:64, (k - 16) * 128 : (k - 15) * 128]

                sc = psA.tile([128, 1024], FP32, tag="ps")
                if ps == 1 and k >= 2:
                    # heater: keeps the PE HAM clock warm; result is
                    # overwritten by the start=True scores matmul below.
                    for _ in range(2):
                        nc.tensor.transpose(
                            out=sc[:, 0:64].bitcast(BF16),
                            in_=identB[:, :],
                            identity=identB[:, :],
                        )
                for c in range(2):
                    q0 = ps * 1024 + c * 512
                    nc.tensor.matmul(
                        out=sc[:, c * 512 : (c + 1) * 512],
                        lhsT=klhs,
                        rhs=qts[:, q0 : q0 + 512],
                        start=True,
                        stop=True,
                    )
                p = pp.tile([128, 1024], BF16)
                nc.scalar.activation(out=p[:, :], in_=sc[:, :], func=Exp, scale=SCALE)
                p_tiles[k] = p

                if k >= 2:
                    _attn(nc, att_ps, vn, p_tiles, k - 2)

            _attn(nc, att_ps, vn, p_tiles, NKT - 2)
            _attn(nc, att_ps, vn, p_tiles, NKT - 1)

            # ship this pass's attn^T + denominators (host normalizes)
            for c in range(2):
                cols = slice(ps * 1024 + c * 512, ps * 1024 + (c + 1) * 512)
                pcols = slice(c * 512, (c + 1) * 512)
                nc.vector.tensor_copy(out=att_sb[:, cols], in_=att_ps[0:65, pcols])
                out_engs[c].dma_start(out=out_d[:, cols], in_=att_sb[:, cols])

    nc.compile()
    _CACHE["nc"] = nc
    return nc


def _attn(nc, att_ps, vn, p_tiles, k):
    p = p_tiles.pop(k)
    for c in range(2):
        nc.tensor.matmul(
            out=att_ps[0:65, c * 512 : (c + 1) * 512],
            lhsT=vn[:, k, :],
            rhs=p[:, c * 512 : (c + 1) * 512],
            start=(k == 0),
            stop=(k == NKT - 1),
            skip_group_check=True,
        )


def _make_in_maps(x, Wq, Wk, Wv):
    import ml_dtypes

    bf16 = ml_dtypes.bfloat16
    xT = np.ascontiguousarray(x.transpose(0, 2, 1)).astype(bf16)  # [B, E, S]
    wt = np.concatenate(
        [Wk.T, Wq.T, Wk.T, Wv.T, Wv.T], axis=1
    ).astype(bf16)  # [E, 320]
    in_maps = []
    for c in range(N_CORES):
        b, h = divmod(c, 2)
        in_maps.append(
            {
                "xt_q": np.ascontiguousarray(xT[b, :, h * HALF : (h + 1) * HALF]),
                "xt_o": np.ascontiguousarray(
                    xT[b, :, (1 - h) * HALF : (2 - h) * HALF]
                ),
                "wt": wt,
            }
        )
    return in_maps


def _run(x, Wq, Wk, Wv, trace=False):
    from concourse.bass_utils import run_bass_kernel_spmd

    nc = _build()
    in_maps = _make_in_maps(x, Wq, Wk, Wv)
    res = run_bass_kernel_spmd(
        nc, in_maps, core_ids=list(range(N_CORES)), trace=trace
    )
    out = np.empty((B, S, D), dtype=np.float32)
    for c in range(N_CORES):
        b, h = divmod(c, 2)
        att = res.results[c]["out"]  # [65, HALF]: attn^T rows + denom row
        out[b, h * HALF : (h + 1) * HALF, :] = (att[0:D] / att[D : D + 1]).T
    return out, res


def kernel(x, Wq, Wk, Wv):
    out, _ = _run(
        np.asarray(x, dtype=np.float32),
        np.asarray(Wq, dtype=np.float32),
        np.asarray(Wk, dtype=np.float32),
        np.asarray(Wv, dtype=np.float32),
    )
    return out
